# revision 7
# baseline (speedup 1.0000x reference)
"""MoE gate (softmax -> top-8 -> renorm + aux-loss stats) on 8 trn2 cores.

Sharding: data-parallel over the flattened token dim (2048 tokens/core).
Each core receives its x-shard pre-transposed to [D, T] so the contraction
dim (D) lies on SBUF partitions, plus the replicated gate weight transposed
to [D, E].

Device kernel (per core):
  - 16 d-chunk matmuls accumulate logits [128 tok, 64 exp] in PSUM
  - ScalarE: exp + row-sum (accum_out) in one instruction
  - VectorE: reciprocal, max (top-8 desc), max_index (exact argmax-8),
    top-8 renorm (sum/recip/mul)
  - pi partial: PE matmul with the reciprocal column stationary:
    [1,64] += r.T @ exp  == column-sums of softmax scores
Host: gathers per-core outputs, computes fi from an index histogram and
the scalar aux loss (64-element math).
"""

import os
import sys

import numpy as np

for _p in ("/opt/trn_rl_repo", "/root/.axon_site/_ro/trn_rl_repo"):
    if os.path.isdir(_p) and _p not in sys.path:
        sys.path.insert(0, _p)

import concourse.bacc as bacc
import concourse.mybir as mybir
import concourse.tile as tile
from concourse import masks
from concourse.bass_utils import run_bass_kernel_spmd

B, S, D = 4, 4096, 2048
E, K = 64, 8
NCORES = 8
N = B * S
T = N // NCORES            # 2048 tokens per core
TB = 512                   # tokens per DMA block
NB = T // TB               # 4 blocks
TT = TB // 128             # 4 token-tiles per block
DC = D // 128              # 16 contraction chunks
AUX_ALPHA = np.float32(0.01)

_cache = {}


def _build_nc():
    f32 = mybir.dt.float32
    u32 = mybir.dt.uint32
    nc = bacc.Bacc(None)
    xt = nc.dram_tensor("xt", [D, T], f32, kind="ExternalInput")
    wt = nc.dram_tensor("wt", [D, E], f32, kind="ExternalInput")
    w8o = nc.dram_tensor("w8o", [T, K], f32, kind="ExternalOutput")
    i8o = nc.dram_tensor("i8o", [T, K], u32, kind="ExternalOutput")
    pio = nc.dram_tensor("pio", [1, E], f32, kind="ExternalOutput")

    with tile.TileContext(nc) as tc:
        with (
            tc.tile_pool(name="wpool", bufs=1) as wpool,
            tc.tile_pool(name="xpool", bufs=32) as xpool,
            tc.tile_pool(name="lpool", bufs=2) as lpool,
            tc.tile_pool(name="epool", bufs=4) as epool,
            tc.tile_pool(name="small", bufs=4) as spool,
            tc.tile_pool(name="acc", bufs=1) as apool,
            tc.tile_pool(name="lgpsum", bufs=2, space="PSUM") as lgpool,
            tc.tile_pool(name="trpsum", bufs=4, space="PSUM") as trpool,
            tc.tile_pool(name="pipsum", bufs=2, space="PSUM") as pipool,
        ):
            # Replicated gate weight, d-chunked: wsb[:, c*E:(c+1)*E] = wT[c]
            wsb = wpool.tile([128, DC * E], f32)
            for c in range(DC):
                nc.sync.dma_start(wsb[:, c * E:(c + 1) * E],
                                  wt[c * 128:(c + 1) * 128, :])
            ident = wpool.tile([64, 64], f32)
            masks.make_identity(nc, ident[:])

            pi_acc = apool.tile([1, E], f32)
            nc.vector.memset(pi_acc[:], 0.0)

            def epilogue(b, lgs):
                # Per 128-token tile: PE transpose back to [tok, exp],
                # exp+rowsum on ACT, top-8 + renorm on DVE, pi partial on PE.
                for t in range(TT):
                    g = b * TT + t
                    tr = trpool.tile([128, E], f32, tag="tr")
                    nc.tensor.transpose(
                        tr[:], lgs[:, t * 128:(t + 1) * 128], ident[:])
                    ex = epool.tile([128, E], f32, tag="ex")
                    esum = spool.tile([128, 1], f32, tag="esum")
                    nc.scalar.activation(
                        ex[:], tr[:], mybir.ActivationFunctionType.Exp,
                        accum_out=esum[:])
                    r = spool.tile([128, 1], f32, tag="r")
                    nc.vector.reciprocal(r[:], esum[:])
                    e8 = spool.tile([128, 8], f32, tag="e8")
                    nc.vector.max(e8[:], ex[:])
                    i8 = spool.tile([128, 8], u32, tag="i8")
                    nc.vector.max_index(i8[:], e8[:], ex[:])
                    s8 = spool.tile([128, 1], f32, tag="s8")
                    nc.vector.reduce_sum(s8[:], e8[:], axis=mybir.AxisListType.X)
                    rs = spool.tile([128, 1], f32, tag="rs")
                    nc.vector.reciprocal(rs[:], s8[:])
                    w8 = spool.tile([128, 8], f32, tag="w8")
                    nc.vector.tensor_scalar_mul(w8[:], e8[:], rs[:])
                    # pi partial: [1,64] = sum_tok r_tok * exp_tok == scores colsum
                    pp = pipool.tile([1, E], f32)
                    nc.tensor.matmul(pp[:], r[:], ex[:], start=True, stop=True)
                    nc.vector.tensor_add(pi_acc[:], pi_acc[:], pp[:])
                    nc.gpsimd.dma_start(w8o[g * 128:(g + 1) * 128, :], w8[:])
                    nc.gpsimd.dma_start(i8o[g * 128:(g + 1) * 128, :], i8[:])

            # Software-pipelined: block b's epilogue is emitted after block
            # b+1's logits matmuls, so PE never stalls on the ACT/DVE chain
            # and stays dense enough to warm up.
            pending = None
            for b in range(NB):
                xcs = []
                for c in range(DC):
                    xc = xpool.tile([128, TB], f32, tag="xc")
                    nc.sync.dma_start(
                        xc[:], xt[c * 128:(c + 1) * 128, b * TB:(b + 1) * TB])
                    xcs.append(xc)
                # logits.T for the whole block: [64 exp, 512 tok].
                # Stationary = weight chunk (64 cols), moving = tokens
                # (512 cols, the fp32 max) so LDWEIGHTS stays off the
                # critical path.
                lg = lgpool.tile([64, TB], f32)
                for c in range(DC):
                    nc.tensor.matmul(
                        lg[:],
                        wsb[:, c * E:(c + 1) * E],
                        xcs[c][:],
                        start=(c == 0),
                        stop=(c == DC - 1),
                    )
                lgs = lpool.tile([64, TB], f32, tag="lgs")
                nc.scalar.copy(lgs[:], lg[:])
                if pending is not None:
                    epilogue(*pending)
                pending = (b, lgs)
            epilogue(*pending)
            nc.sync.dma_start(pio[:], pi_acc[:])
    nc.compile()
    return nc


def _get_nc():
    if "nc" not in _cache:
        _cache["nc"] = _build_nc()
    return _cache["nc"]


def _make_in_maps(x, weight):
    xf = np.ascontiguousarray(
        np.asarray(x, dtype=np.float32).reshape(N, D))
    wT = np.ascontiguousarray(np.asarray(weight, dtype=np.float32).T)
    in_maps = []
    for cid in range(NCORES):
        xTs = np.ascontiguousarray(xf[cid * T:(cid + 1) * T].T)
        in_maps.append({"xt": xTs, "wt": wT})
    return in_maps


def _run_device(x, weight, trace=False, **kw):
    nc = _get_nc()
    in_maps = _make_in_maps(x, weight)
    return run_bass_kernel_spmd(nc, in_maps, list(range(NCORES)), trace=trace, **kw)


def _assemble(results):
    w8 = np.concatenate([results[c]["w8o"] for c in range(NCORES)], axis=0)
    i8 = np.concatenate([results[c]["i8o"] for c in range(NCORES)],
                        axis=0).astype(np.int32)
    pi = np.sum(np.stack([results[c]["pio"][0] for c in range(NCORES)]),
                axis=0, dtype=np.float32) / np.float32(N)
    counts = np.bincount(i8.reshape(-1), minlength=E).astype(np.float32)
    fi = counts / np.float32(N * K) * np.float32(E)
    aux = np.float32(np.sum(pi * fi, dtype=np.float32) * AUX_ALPHA)
    return w8, i8, aux


def kernel(x, weight):
    res = _run_device(x, weight, trace=False)
    return _assemble(res.results)


# revision 11
# speedup vs baseline: 1.2504x; 1.2504x over previous
"""MoE gate (softmax -> top-8 -> renorm + aux-loss stats) on 8 trn2 cores.

Sharding: data-parallel over the flattened token dim (2048 tokens/core).
Each core receives its x-shard pre-transposed to [D, T] so the contraction
dim (D) lies on SBUF partitions, plus the replicated gate weight transposed
to [D, E].

Device kernel (per core):
  - 16 d-chunk matmuls accumulate logits [128 tok, 64 exp] in PSUM
  - ScalarE: exp + row-sum (accum_out) in one instruction
  - VectorE: reciprocal, max (top-8 desc), max_index (exact argmax-8),
    top-8 renorm (sum/recip/mul)
  - pi partial: PE matmul with the reciprocal column stationary:
    [1,64] += r.T @ exp  == column-sums of softmax scores
Host: gathers per-core outputs, computes fi from an index histogram and
the scalar aux loss (64-element math).
"""

import os
import sys

import numpy as np

for _p in ("/opt/trn_rl_repo", "/root/.axon_site/_ro/trn_rl_repo"):
    if os.path.isdir(_p) and _p not in sys.path:
        sys.path.insert(0, _p)

import concourse.bacc as bacc
import concourse.mybir as mybir
import concourse.tile as tile
from concourse import masks
from concourse.bass_utils import run_bass_kernel_spmd

B, S, D = 4, 4096, 2048
E, K = 64, 8
NCORES = 8
N = B * S
T = N // NCORES            # 2048 tokens per core
TB = 512                   # tokens per DMA block
NB = T // TB               # 4 blocks
TT = TB // 128             # 4 token-tiles per block
DC = D // 128              # 16 contraction chunks
AUX_ALPHA = np.float32(0.01)

_cache = {}


def _build_nc():
    f32 = mybir.dt.float32
    u32 = mybir.dt.uint32
    nc = bacc.Bacc(None)
    xt = nc.dram_tensor("xt", [D, T], f32, kind="ExternalInput")
    wt = nc.dram_tensor("wt", [D, E], f32, kind="ExternalInput")
    w8o = nc.dram_tensor("w8o", [T, K], f32, kind="ExternalOutput")
    i8o = nc.dram_tensor("i8o", [T, K], u32, kind="ExternalOutput")
    pio = nc.dram_tensor("pio", [1, E], f32, kind="ExternalOutput")

    with tile.TileContext(nc) as tc:
        with (
            tc.tile_pool(name="wpool", bufs=1) as wpool,
            tc.tile_pool(name="xpool", bufs=32) as xpool,
            tc.tile_pool(name="lpool", bufs=2) as lpool,
            tc.tile_pool(name="epool", bufs=4) as epool,
            tc.tile_pool(name="opool", bufs=2) as opool,
            tc.tile_pool(name="small", bufs=4) as spool,
            tc.tile_pool(name="acc", bufs=1) as apool,
            tc.tile_pool(name="lgpsum", bufs=2, space="PSUM") as lgpool,
            tc.tile_pool(name="trpsum", bufs=3, space="PSUM") as trpool,
            tc.tile_pool(name="pipsum", bufs=1, space="PSUM") as pipool,
        ):
            # Replicated gate weight, d-chunked: wsb[:, c*E:(c+1)*E] = wT[c].
            # One coalesced DMA on the scalar HWDGE ring so the x stream on
            # the sync ring is not head-of-line blocked.
            wsb = wpool.tile([128, DC * E], f32)
            nc.scalar.dma_start(
                wsb[:].rearrange("p (c e) -> p c e", c=DC),
                wt.rearrange("(c p) e -> p c e", p=128))
            ident = wpool.tile([64, 64], f32)
            masks.make_identity(nc, ident[:])
            ones = wpool.tile([128, 1], f32)
            nc.vector.memset(ones[:], 1.0)

            # running sum of softmax scores over this core's tokens
            sacc = apool.tile([128, E], f32)
            nc.vector.memset(sacc[:], 0.0)

            def epilogue(b, lgs):
                # Per 128-token tile: PE transpose back to [tok, exp],
                # exp+rowsum on ACT, top-8 + renorm on DVE, score accum on DVE.
                wstage = opool.tile([128, TT * K], f32, tag="wstage")
                istage = opool.tile([128, TT * K], u32, tag="istage")
                for t in range(TT):
                    tr = trpool.tile([128, E], f32, tag="tr")
                    nc.tensor.transpose(
                        tr[:], lgs[:, t * 128:(t + 1) * 128], ident[:])
                    ex = epool.tile([128, E], f32, tag="ex")
                    esum = spool.tile([128, 1], f32, tag="esum")
                    nc.scalar.activation(
                        ex[:], tr[:], mybir.ActivationFunctionType.Exp,
                        accum_out=esum[:])
                    r = spool.tile([128, 1], f32, tag="r")
                    nc.vector.reciprocal(r[:], esum[:])
                    e8 = spool.tile([128, 8], f32, tag="e8")
                    nc.vector.max(e8[:], ex[:])
                    nc.vector.max_index(istage[:, t * K:(t + 1) * K], e8[:], ex[:])
                    s8 = spool.tile([128, 1], f32, tag="s8")
                    nc.vector.reduce_sum(s8[:], e8[:], axis=mybir.AxisListType.X)
                    rs = spool.tile([128, 1], f32, tag="rs")
                    nc.vector.reciprocal(rs[:], s8[:])
                    nc.vector.tensor_scalar_mul(
                        wstage[:, t * K:(t + 1) * K], e8[:], rs[:])
                    s = epool.tile([128, E], f32, tag="s")
                    nc.vector.tensor_scalar_mul(s[:], ex[:], r[:])
                    nc.vector.tensor_add(sacc[:], sacc[:], s[:])
                nc.scalar.dma_start(
                    w8o[b * TB:(b + 1) * TB, :].rearrange(
                        "(t p) k -> p t k", p=128),
                    wstage[:].rearrange("p (t k) -> p t k", t=TT))
                nc.scalar.dma_start(
                    i8o[b * TB:(b + 1) * TB, :].rearrange(
                        "(t p) k -> p t k", p=128),
                    istage[:].rearrange("p (t k) -> p t k", t=TT))

            # Software-pipelined: block b's epilogue is emitted after block
            # b+1's logits matmuls, so PE never stalls on the ACT/DVE chain.
            pending = None
            for b in range(NB):
                xcs = []
                for c in range(DC):
                    xc = xpool.tile([128, TB], f32, tag="xc")
                    nc.sync.dma_start(
                        xc[:], xt[c * 128:(c + 1) * 128, b * TB:(b + 1) * TB])
                    xcs.append(xc)
                # logits.T for the whole block: [64 exp, 512 tok], computed as
                # two concurrent column-group matmul streams (the 64-column
                # stationary only fills half the PE array): even chunks
                # accumulate into psum rows 0:64, odd chunks into 64:128.
                lg = lgpool.tile([128, TB], f32)
                for ci in range(0, DC, 2):
                    nc.tensor.matmul(
                        lg[0:64, :],
                        wsb[:, ci * E:(ci + 1) * E],
                        xcs[ci][:],
                        start=(ci == 0),
                        stop=(ci == DC - 2),
                        tile_position=(0, 0),
                        skip_group_check=True,
                    )
                    nc.tensor.matmul(
                        lg[64:128, :],
                        wsb[:, (ci + 1) * E:(ci + 2) * E],
                        xcs[ci + 1][:],
                        start=(ci == 0),
                        stop=(ci == DC - 2),
                        tile_position=(0, 64),
                        skip_group_check=True,
                    )
                lgt = lpool.tile([64, TB], f32, tag="lgt")
                nc.scalar.copy(lgt[:], lg[64:128, :])
                lgs = lpool.tile([64, TB], f32, tag="lgs")
                nc.vector.tensor_add(lgs[:], lg[0:64, :], lgt[:])
                if pending is not None:
                    epilogue(*pending)
                pending = (b, lgs)
            epilogue(*pending)
            # pi partial for the whole core: [1,64] = ones.T @ sacc
            pp = pipool.tile([1, E], f32)
            nc.tensor.matmul(pp[:], ones[:], sacc[:], start=True, stop=True)
            pi_sb = spool.tile([1, E], f32, tag="pisb")
            nc.vector.tensor_copy(pi_sb[:], pp[:])
            nc.sync.dma_start(pio[:], pi_sb[:])
    nc.compile()
    return nc


def _get_nc():
    if "nc" not in _cache:
        _cache["nc"] = _build_nc()
    return _cache["nc"]


def _make_in_maps(x, weight):
    xf = np.ascontiguousarray(
        np.asarray(x, dtype=np.float32).reshape(N, D))
    wT = np.ascontiguousarray(np.asarray(weight, dtype=np.float32).T)
    in_maps = []
    for cid in range(NCORES):
        xTs = np.ascontiguousarray(xf[cid * T:(cid + 1) * T].T)
        in_maps.append({"xt": xTs, "wt": wT})
    return in_maps


def _run_device(x, weight, trace=False, **kw):
    nc = _get_nc()
    in_maps = _make_in_maps(x, weight)
    return run_bass_kernel_spmd(nc, in_maps, list(range(NCORES)), trace=trace, **kw)


def _assemble(results):
    w8 = np.concatenate([results[c]["w8o"] for c in range(NCORES)], axis=0)
    i8 = np.concatenate([results[c]["i8o"] for c in range(NCORES)],
                        axis=0).astype(np.int32)
    pi = np.sum(np.stack([results[c]["pio"][0] for c in range(NCORES)]),
                axis=0, dtype=np.float32) / np.float32(N)
    counts = np.bincount(i8.reshape(-1), minlength=E).astype(np.float32)
    fi = counts / np.float32(N * K) * np.float32(E)
    aux = np.float32(np.sum(pi * fi, dtype=np.float32) * AUX_ALPHA)
    return w8, i8, aux


def kernel(x, weight):
    res = _run_device(x, weight, trace=False)
    return _assemble(res.results)
